# revision 1
# baseline (speedup 1.0000x reference)
"""Trainium2 Bass kernel for nn_AutoEncoderLoss (two-level segment-mean MSE).

Strategy
--------
batch_index is sorted, so the N points split into `num_batches` contiguous
runs. The host finds the 32 run boundaries (np.searchsorted - O(B log N)) and
shards *whole batches* across the 8 cores (4 batches/core, sizes are
near-identical). Each batch range is laid out as a [128, T_pad] tile
(contiguous per partition), padded with clabel=255 (out-of-range -> one-hot
all zero) and reco=target=0.

On each core, for every batch range we compute a 128-bin weighted histogram
(cluster sums of (reco-target)^2, and counts) with a factored one-hot:
  h = clabel >> 3 (16 values), l = clabel & 7 (8 values)
  DVE builds bin-major "slabs" with constant-scalar compares (fast 4x mode):
    16x (h==H) bf16, 8x (l==L) bf16, 8x (l==L)*v bf16
  PE multiplies hi-slabs against lo-slabs 8 point-columns at a time:
    lhsT[128, 8*16] (8 chunks' hi one-hots), rhs[128, 8*16] (lo cnt|val),
    accumulating in PSUM. The 8 diagonal [16,16] blocks hold
    [counts | sums] per (H, L); off-diagonal blocks are ignored junk.
PSUM banks (one per batch range) are dumped to DRAM; the host folds the
8 diagonal blocks, assembles the [32, 128] segment sums/counts and does the
final O(B*C) masked-mean reduction.
"""

import math
import numpy as np
from contextlib import ExitStack

NCORES = 8
HI = 16  # hi one-hot bins (clabel >> 3)
LOB = 8  # lo one-hot bins (clabel & 7)
GROUP = 8  # point-columns per matmul (GROUP*HI = 128 = max stationary cols)
import os as _os
T_TILE = int(_os.environ.get("K_T_TILE", "640"))  # SBUF tile width
LOVAL_MODE = _os.environ.get("K_LOVAL", "mul")  # "mul" | "stt"
PAD_LABEL = 255  # out-of-range label: h=31 matches no hi bin
RB = 12582912.0  # 1.5 * 2**23, fp32 round-to-int bias

_prog_cache = {}
_last_run = {}  # stashed (nc, in_maps) from the latest kernel() call


def profile_hw(np_inputs=None, k1=4, k2=1004, pairs=10, verbose=False):
    """Measure steady-state HW ns per kernel iteration.

    Runs two hardware-loop variants (k1/k2 repeats of the full compute,
    Internal-DRAM inputs so no transfers) in interleaved pairs; the median
    of per-pair wall-clock differences divided by (k2-k1) cancels dispatch
    overhead and is robust to the time-shared device's slow patches.
    """
    import time
    from concourse.bass_utils import run_bass_kernel_spmd
    if not _last_run and np_inputs is not None:
        kernel(**np_inputs)
    T_pad, R = _last_run["key"]

    ncs = {}
    for k in (k1, k2):
        ck = ("prof", T_pad, R, k, "full")
        if ck not in _prog_cache:
            _prog_cache[ck] = _build_program(T_pad, R, repeat=k,
                                             internal_inputs=True)
        ncs[k] = _prog_cache[ck]

    def one(k):
        t0 = time.time()
        run_bass_kernel_spmd(ncs[k], [{} for _ in range(NCORES)],
                             list(range(NCORES)))
        return time.time() - t0

    one(k1)  # warm both NEFFs
    one(k2)
    diffs = []
    for _ in range(pairs):
        try:
            ta = one(k1)
            tb = one(k2)
        except Exception:  # transient device flake: skip pair
            time.sleep(2)
            continue
        diffs.append((tb - ta) / (k2 - k1) * 1e9)
    diffs.sort()
    if verbose:
        print("pair diffs (ns/iter):", [f"{d:.0f}" for d in diffs])
    return diffs[len(diffs) // 2] if diffs else float("nan")


def profile_stages(np_inputs=None, k1=4, k2=104, samples=4):
    """Per-stage steady-state times (us): dma, +dve, +act-repack, full."""
    if not _last_run and np_inputs is not None:
        kernel(**np_inputs)
    out = {}
    for stage in ("dma", "dve", "act", "full"):
        import importlib
        t1 = _timed_prof(k1, stage, samples)
        t2 = _timed_prof(k2, stage, samples)
        out[stage] = (t2 - t1) / (k2 - k1) * 1e6
    return out


def _timed_prof(k, stage, samples):
    import time
    from concourse.bass_utils import run_bass_kernel_spmd
    T_pad, R = _last_run["key"]
    ck = ("prof", T_pad, R, k, stage)
    if ck not in _prog_cache:
        _prog_cache[ck] = _build_program(T_pad, R, repeat=k,
                                         internal_inputs=True, stage=stage)
    nc = _prog_cache[ck]
    best = float("inf")
    for _ in range(samples):
        t0 = time.time()
        run_bass_kernel_spmd(nc, [{} for _ in range(NCORES)],
                             list(range(NCORES)))
        best = min(best, time.time() - t0)
    return best


def _build_program(T_pad, R, repeat=None, internal_inputs=False, stage="full"):
    """Build + compile the SPMD bass program for R ranges of T_pad columns.

    repeat: wrap the whole compute in a hardware For_i loop (profiling).
    internal_inputs: inputs become Internal DRAM scratch (garbage data, no
    host transfer) - timing is data-independent, used only for profiling.
    """
    import concourse.tile as tile
    from concourse import bacc, mybir

    f32 = mybir.dt.float32
    bf16 = mybir.dt.bfloat16
    i32 = mybir.dt.int32
    AT = mybir.ActivationFunctionType
    OP = mybir.AluOpType

    nc = bacc.Bacc("TRN2", target_bir_lowering=False, debug=False,
                   num_devices=NCORES)
    in_kind = "Internal" if internal_inputs else "ExternalInput"
    rec = nc.dram_tensor("rec", [128, R * T_pad], f32, kind=in_kind).ap()
    tar = nc.dram_tensor("tar", [128, R * T_pad], f32, kind=in_kind).ap()
    lab = nc.dram_tensor("lab", [128, R * T_pad], i32, kind=in_kind).ap()
    out = nc.dram_tensor("out", [128, R * 128], f32, kind="ExternalOutput").ap()

    tiles = []
    t0 = 0
    while t0 < T_pad:
        tw = min(T_TILE, T_pad - t0)
        tiles.append((t0, tw))
        t0 += tw
    n_mm = T_pad // GROUP  # one matmul per GROUP point-columns per range

    with tile.TileContext(nc) as tc, ExitStack() as ctx:
        io_pool = ctx.enter_context(tc.tile_pool(name="io", bufs=2))
        tmp_pool = ctx.enter_context(tc.tile_pool(name="tmp", bufs=2))
        slab_pool = ctx.enter_context(tc.tile_pool(name="slab", bufs=2))
        psum_pool = ctx.enter_context(tc.tile_pool(name="psum", bufs=1, space="PSUM"))
        out_pool = ctx.enter_context(tc.tile_pool(name="outp", bufs=2))

        psums = [psum_pool.tile([128, 128], f32, tag=f"ps{r}", name=f"ps{r}")
                 for r in range(R)] if stage == "full" else [None] * R

        if repeat is not None:
            ctx.enter_context(tc.For_i(0, repeat, 1))

        for r in range(R):
            base = r * T_pad
            mm_i = 0
            for (t0, tw) in tiles:
                rec_t = io_pool.tile([128, tw], f32, tag="rec")
                nc.sync.dma_start(out=rec_t[:], in_=rec[:, base + t0:base + t0 + tw])
                tar_t = io_pool.tile([128, tw], f32, tag="tar")
                nc.sync.dma_start(out=tar_t[:], in_=tar[:, base + t0:base + t0 + tw])
                lab_t = io_pool.tile([128, tw], i32, tag="lab")
                nc.sync.dma_start(out=lab_t[:], in_=lab[:, base + t0:base + t0 + tw])
                if stage == "dma":
                    continue

                # h = floor(clabel/8) via fp32 round-to-nearest bias trick:
                # RN(c*0.125 - 0.4375) == floor(c/8) exactly for c in [0,256).
                # (c - 3.5)*0.125 first; then +RB forces integer rounding, -RB
                # recovers h. RB +/- offsets must stay separate ops: ulp(RB)=1.
                hf = tmp_pool.tile([128, tw], f32, tag="hf")
                nc.vector.tensor_scalar(hf[:], lab_t[:], -3.5, 0.125,
                                        OP.add, OP.mult)
                hb = tmp_pool.tile([128, tw], bf16, tag="hb")
                nc.vector.tensor_scalar(hb[:], hf[:], RB, -RB, OP.add, OP.add)
                # l = clabel - 8*h
                lb = tmp_pool.tile([128, tw], bf16, tag="lb")
                nc.vector.scalar_tensor_tensor(lb[:], hb[:], -8.0, lab_t[:],
                                               OP.mult, OP.add)
                # v = (rec - tar)^2 (bf16), square on ScalarE
                d_t = tmp_pool.tile([128, tw], f32, tag="d")
                nc.vector.tensor_sub(d_t[:], rec_t[:], tar_t[:])
                vb = tmp_pool.tile([128, tw], bf16, tag="vb")
                nc.scalar.activation(vb[:], d_t[:], AT.Square)

                # bin-major slabs
                hic = slab_pool.tile([128, HI, tw], bf16, tag="hic")
                for h in range(HI):
                    nc.vector.tensor_scalar(hic[:, h, :], hb[:], float(h), None,
                                            OP.is_equal)
                lot = slab_pool.tile([128, 2 * LOB, tw], bf16, tag="lot")
                for l in range(LOB):
                    nc.vector.tensor_scalar(lot[:, l, :], lb[:], float(l), None,
                                            OP.is_equal)
                # (l==L)*v via tensor_tensor on the count slab: TT bf16 runs
                # 2x while fused scalar_tensor_tensor measures only 1x
                for l in range(LOB):
                    if LOVAL_MODE == "mul":
                        nc.vector.tensor_mul(lot[:, LOB + l, :], lot[:, l, :],
                                             vb[:])
                    else:
                        nc.vector.scalar_tensor_tensor(lot[:, LOB + l, :],
                                                       lb[:], float(l), vb[:],
                                                       OP.is_equal, OP.mult)

                if stage == "dve":
                    continue
                # repack hi-slabs to t-major on ScalarE: hic2[p, t*16+H]
                # -> contiguous 128-col LDWEIGHTS slices (FWL-eligible)
                hic2 = slab_pool.tile([128, tw, HI], bf16, tag="hic2")
                nc.scalar.copy(hic2[:], hic[:].transpose([0, 2, 1]))
                if stage == "act":
                    continue

                for g in range(tw // GROUP):
                    lhsT = hic2[:, g * GROUP:(g + 1) * GROUP, :]  # [128,8,16] contig
                    rhs = lot[:, :, g * GROUP:(g + 1) * GROUP].transpose([0, 2, 1])
                    nc.tensor.matmul(psums[r][:], lhsT, rhs,
                                     start=(mm_i == 0), stop=(mm_i == n_mm - 1))
                    mm_i += 1

            if stage == "full":
                ob = out_pool.tile([128, 128], f32, tag="ob")
                nc.vector.tensor_copy(ob[:], psums[r][:])
                nc.sync.dma_start(out=out[:, r * 128:(r + 1) * 128], in_=ob[:])

    nc.compile()
    return nc


def kernel(reco, target, clabel, batch_index, num_batches, num_clusters):
    from concourse.bass_utils import run_bass_kernel_spmd

    B = int(num_batches)
    C = int(num_clusters)
    assert C == HI * LOB, f"kernel hardcoded for 128 clusters, got {C}"
    assert B % NCORES == 0, f"num_batches {B} not divisible by {NCORES}"
    R = B // NCORES

    reco = np.ascontiguousarray(np.asarray(reco, dtype=np.float32).reshape(-1))
    target = np.ascontiguousarray(np.asarray(target, dtype=np.float32).reshape(-1))
    clabel = np.asarray(clabel).astype(np.int32).reshape(-1)
    batch_index = np.asarray(batch_index).reshape(-1)
    N = reco.shape[0]

    # host: batch run boundaries (batch_index is sorted)
    bnd = np.searchsorted(batch_index, np.arange(B + 1), side="left")
    lens = np.diff(bnd)
    t_len = (lens + 127) // 128  # columns per batch
    T_pad = int(-(-int(t_len.max()) // GROUP) * GROUP)
    T_pad = max(T_pad, GROUP)

    key = (T_pad, R)
    if key not in _prog_cache:
        _prog_cache[key] = _build_program(T_pad, R)
    nc = _prog_cache[key]

    # build per-core input buffers
    in_maps = []
    for m in range(NCORES):
        rec_buf = np.zeros((128, R * T_pad), dtype=np.float32)
        tar_buf = np.zeros((128, R * T_pad), dtype=np.float32)
        lab_buf = np.full((128, R * T_pad), PAD_LABEL, dtype=np.int32)
        for r in range(R):
            b = m * R + r
            s, e = int(bnd[b]), int(bnd[b + 1])
            n = e - s
            if n == 0:
                continue
            tpb = (n + 127) // 128  # columns used by this batch
            block = np.zeros(128 * tpb, dtype=np.float32)
            block[:n] = reco[s:e]
            rec_buf[:, r * T_pad:r * T_pad + tpb] = block.reshape(128, tpb)
            block = np.zeros(128 * tpb, dtype=np.float32)
            block[:n] = target[s:e]
            tar_buf[:, r * T_pad:r * T_pad + tpb] = block.reshape(128, tpb)
            lblock = np.full(128 * tpb, PAD_LABEL, dtype=np.int32)
            lblock[:n] = clabel[s:e]
            lab_buf[:, r * T_pad:r * T_pad + tpb] = lblock.reshape(128, tpb)
        in_maps.append({"rec": rec_buf, "tar": tar_buf, "lab": lab_buf})

    _last_run["nc"] = nc
    _last_run["in_maps"] = in_maps
    _last_run["key"] = key
    res = None
    last_err = None
    for _attempt in range(3):  # the device occasionally faults transiently
        try:
            res = run_bass_kernel_spmd(nc, in_maps, list(range(NCORES)))
            break
        except Exception as e:  # noqa: BLE001
            last_err = e
            import time as _time
            _time.sleep(2.0)
    if res is None:
        raise last_err

    # host: fold diagonal blocks -> [B, C] sums/counts, then final reduction
    counts = np.zeros((B, C), dtype=np.float64)
    sums = np.zeros((B, C), dtype=np.float64)
    jj = np.arange(GROUP)
    for m in range(NCORES):
        o = res.results[m]["out"].astype(np.float64)  # [128, R*128]
        for r in range(R):
            b = m * R + r
            P = o[:, r * 128:(r + 1) * 128]
            # real data sits in the 8 diagonal [16,16] blocks (j==j')
            blocks = P.reshape(GROUP, HI, GROUP, 2 * LOB)[jj, :, jj, :]
            folded = blocks.sum(axis=0)  # [16 (H), 16 (L|8+L)]
            counts[b] = folded[:, :LOB].reshape(C)
            sums[b] = folded[:, LOB:].reshape(C)

    present = counts > 0
    means = np.where(present, sums / np.where(present, counts, 1.0), 0.0)
    pmask = present.astype(np.float64)
    n_clusters_b = pmask.sum(axis=1)
    b_present = n_clusters_b > 0
    batch_loss = (means * pmask).sum(axis=1) / np.where(b_present, n_clusters_b, 1.0)
    n_b = b_present.sum()
    loss = np.where(b_present, batch_loss, 0.0).sum() / max(n_b, 1)
    return np.float32(loss)



# revision 2
# speedup vs baseline: 1.0218x; 1.0218x over previous
"""Trainium2 Bass kernel v2 for nn_AutoEncoderLoss (two-level segment-mean MSE).

Pipeline per [128, tw] tile (points along partitions, t along free dim):
  inputs: rec bf16, tar bf16, lab u8 (5 B/point of HBM traffic)
  DVE:  wb  = bf16(lab + 2076.5)            -> 2080 + 8*(lab>>3)   [TS]
        lb  = (wb - 2080) - lab             -> -(lab & 7)          [STT]
          (or lab mod 8 -> +l when K2_LO=mod)
        d   = rec - tar                                             [TT 2x]
        hi slabs: 16x is_equal(wb, 2080+8H) -> bin-major hic       [TS 4x]
        lo slabs:  8x is_equal(lb, -L)      -> lot[0:8]            [TS 4x]
        val slabs: 8x lot[8+l] = lot[l] * vb                       [TT 2x]
  Act:  vb  = Square(d) -> bf16
  DVE:  hic3 = block-transpose copy of hic -> [g][h][t8] stationary layout
  PE:   per 8-col group g: matmul(psum[r], hic3[g] [128,128], lot[.,g-cols]
        [128, 8t x 16cv]) accumulating counts|sums per (H, L).
Host folds psum diagonal blocks -> [B, C] counts/sums -> masked two-level mean.

Label pad = 255: h=31 matches no hi bin, so pads contribute nothing.
"""

import os as _os
import numpy as np
from contextlib import ExitStack

NCORES = 8
HI = 16
LOB = 8
GROUP = 8
T_TILE = 640
LO_MODE = "stt"
WB_ENGINE = "dve"
HI_LAYOUT = "blk_act"
PAD_LABEL = 255

_prog_cache = {}
_last_run = {}


def _build_program(T_pad, R, repeat=None, internal_inputs=False, stage="full"):
    import concourse.tile as tile
    from concourse import bacc, mybir

    f32 = mybir.dt.float32
    bf16 = mybir.dt.bfloat16
    u8 = mybir.dt.uint8
    AT = mybir.ActivationFunctionType
    OP = mybir.AluOpType

    nc = bacc.Bacc("TRN2", target_bir_lowering=False, debug=False,
                   num_devices=NCORES)
    in_kind = "Internal" if internal_inputs else "ExternalInput"
    rec = nc.dram_tensor("rec", [128, R * T_pad], bf16, kind=in_kind).ap()
    tar = nc.dram_tensor("tar", [128, R * T_pad], bf16, kind=in_kind).ap()
    lab = nc.dram_tensor("lab", [128, R * T_pad], u8, kind=in_kind).ap()
    out = nc.dram_tensor("out", [128, R * 128], f32, kind="ExternalOutput").ap()

    tiles = []
    t0 = 0
    while t0 < T_pad:
        tw = min(T_TILE, T_pad - t0)
        tiles.append((t0, tw))
        t0 += tw
    n_mm = T_pad // GROUP

    with tile.TileContext(nc) as tc, ExitStack() as ctx:
        NB = 2
        io_pool = ctx.enter_context(tc.tile_pool(name="io", bufs=NB))
        tmp_pool = ctx.enter_context(tc.tile_pool(name="tmp", bufs=NB))
        slab_pool = ctx.enter_context(tc.tile_pool(name="slab", bufs=NB))
        psum_pool = ctx.enter_context(tc.tile_pool(name="psum", bufs=1, space="PSUM"))
        out_pool = ctx.enter_context(tc.tile_pool(name="outp", bufs=2))

        psums = [psum_pool.tile([128, 128], f32, tag=f"ps{r}", name=f"ps{r}")
                 for r in range(R)] if stage == "full" else [None] * R

        if repeat is not None:
            ctx.enter_context(tc.For_i(0, repeat, 1))

        for r in range(R):
            base = r * T_pad
            mm_i = 0
            for (t0, tw) in tiles:
                gd = tw // GROUP
                rec_t = io_pool.tile([128, tw], bf16, tag="rec")
                nc.sync.dma_start(out=rec_t[:], in_=rec[:, base + t0:base + t0 + tw])
                tar_t = io_pool.tile([128, tw], bf16, tag="tar")
                nc.sync.dma_start(out=tar_t[:], in_=tar[:, base + t0:base + t0 + tw])
                lab_t = io_pool.tile([128, tw], u8, tag="lab")
                nc.sync.dma_start(out=lab_t[:], in_=lab[:, base + t0:base + t0 + tw])
                if stage == "dma":
                    continue
                # cumulative sub-stages for in-situ cost attribution:
                # dma < pre < hi < lo < mul < rep(=dve) < full
                want = {"pre": 1, "hi": 2, "lo": 3, "mul": 4, "rep": 5,
                        "dve": 5, "full": 9}[stage]

                # wb = 2080 + 8h  (bf16 RN in [2048,4096) floors to x8 grid)
                wb = tmp_pool.tile([128, tw], bf16, tag="wb")
                if WB_ENGINE == "act":
                    nc.scalar.activation(wb[:], lab_t[:], AT.Identity,
                                         bias=2076.5, scale=1.0)
                else:
                    nc.vector.tensor_scalar(wb[:], lab_t[:], 2076.5, None, OP.add)
                # lb = -(l) via STT
                lb = tmp_pool.tile([128, tw], bf16, tag="lb")
                nc.vector.scalar_tensor_tensor(lb[:], wb[:], -2080.0,
                                               lab_t[:], OP.add, OP.subtract)
                lo_const = lambda l: float(-l)
                # d and vb = d^2
                d_t = tmp_pool.tile([128, tw], bf16, tag="d")
                nc.vector.tensor_sub(d_t[:], rec_t[:], tar_t[:])
                vb = tmp_pool.tile([128, tw], bf16, tag="vb")
                nc.scalar.activation(vb[:], d_t[:], AT.Square)
                if want < 2:
                    continue

                # hi slabs
                if HI_LAYOUT == "strided_ld":
                    hic = slab_pool.tile([128, HI, tw], bf16, tag="hic")
                    for h in range(HI):
                        nc.vector.tensor_scalar(hic[:, h, :], wb[:],
                                                2080.0 + 8 * h, None, OP.is_equal)
                    lhs_of = lambda g: hic[:, :, g * GROUP:(g + 1) * GROUP]
                else:
                    hic = slab_pool.tile([128, HI, tw], bf16, tag="hic")
                    for h in range(HI):
                        nc.vector.tensor_scalar(hic[:, h, :], wb[:],
                                                2080.0 + 8 * h, None, OP.is_equal)
                if want < 3:
                    continue

                # lo slabs: counts
                lot = slab_pool.tile([128, 2 * LOB, tw], bf16, tag="lot")
                for l in range(LOB):
                    nc.vector.tensor_scalar(lot[:, l, :], lb[:], lo_const(l),
                                            None, OP.is_equal)
                if want < 4:
                    continue
                # val slabs
                for l in range(LOB):
                    nc.vector.tensor_mul(lot[:, LOB + l, :], lot[:, l, :], vb[:])
                if want < 5:
                    continue

                # stationary repack (unless strided_ld reads hic directly)
                if HI_LAYOUT != "strided_ld":
                    hic3 = slab_pool.tile([128, gd * HI * GROUP], bf16, tag="hic3")
                    src = hic[:].rearrange("p h (g e) -> p g h e", e=GROUP)
                    dst = hic3[:].rearrange("p (g h e) -> p g h e",
                                            g=gd, h=HI, e=GROUP)
                    if HI_LAYOUT == "blk_act":
                        nc.scalar.copy(dst, src)
                    else:
                        nc.vector.tensor_copy(dst, src)
                    h3v = hic3[:].rearrange("p (g h e) -> p g h e",
                                            g=gd, h=HI, e=GROUP)
                    lhs_of = lambda g: h3v[:, g, :, :]

                if want < 9:
                    continue

                for g in range(gd):
                    rhs = lot[:, :, g * GROUP:(g + 1) * GROUP].transpose([0, 2, 1])
                    nc.tensor.matmul(psums[r][:], lhs_of(g), rhs,
                                     start=(mm_i == 0), stop=(mm_i == n_mm - 1))
                    mm_i += 1

            if stage == "full":
                ob = out_pool.tile([128, 128], f32, tag="ob")
                nc.vector.tensor_copy(ob[:], psums[r][:])
                nc.sync.dma_start(out=out[:, r * 128:(r + 1) * 128], in_=ob[:])

    nc.compile()
    return nc


def kernel(reco, target, clabel, batch_index, num_batches, num_clusters):
    import ml_dtypes
    from concourse.bass_utils import run_bass_kernel_spmd

    B = int(num_batches)
    C = int(num_clusters)
    assert C == HI * LOB, f"kernel hardcoded for 128 clusters, got {C}"
    assert B % NCORES == 0, f"num_batches {B} not divisible by {NCORES}"
    R = B // NCORES

    reco = np.ascontiguousarray(np.asarray(reco, dtype=np.float32).reshape(-1))
    target = np.ascontiguousarray(np.asarray(target, dtype=np.float32).reshape(-1))
    clabel = np.asarray(clabel).astype(np.uint8).reshape(-1)
    batch_index = np.asarray(batch_index).reshape(-1)
    N = reco.shape[0]

    bnd = np.searchsorted(batch_index, np.arange(B + 1), side="left")
    t_len = (np.diff(bnd) + 127) // 128
    T_pad = int(-(-int(t_len.max()) // GROUP) * GROUP)
    T_pad = max(T_pad, GROUP)

    key = (T_pad, R)
    if key not in _prog_cache:
        _prog_cache[key] = _build_program(T_pad, R)
    nc = _prog_cache[key]

    in_maps = []
    for m in range(NCORES):
        rec_buf = np.zeros((128, R * T_pad), dtype=ml_dtypes.bfloat16)
        tar_buf = np.zeros((128, R * T_pad), dtype=ml_dtypes.bfloat16)
        lab_buf = np.full((128, R * T_pad), PAD_LABEL, dtype=np.uint8)
        for r in range(R):
            b = m * R + r
            s, e = int(bnd[b]), int(bnd[b + 1])
            n = e - s
            if n == 0:
                continue
            tpb = (n + 127) // 128
            block = np.zeros(128 * tpb, dtype=np.float32)
            block[:n] = reco[s:e]
            rec_buf[:, r * T_pad:r * T_pad + tpb] = \
                block.astype(ml_dtypes.bfloat16).reshape(128, tpb)
            block = np.zeros(128 * tpb, dtype=np.float32)
            block[:n] = target[s:e]
            tar_buf[:, r * T_pad:r * T_pad + tpb] = \
                block.astype(ml_dtypes.bfloat16).reshape(128, tpb)
            lblock = np.full(128 * tpb, PAD_LABEL, dtype=np.uint8)
            lblock[:n] = clabel[s:e]
            lab_buf[:, r * T_pad:r * T_pad + tpb] = lblock.reshape(128, tpb)
        in_maps.append({"rec": rec_buf, "tar": tar_buf, "lab": lab_buf})

    _last_run["nc"] = nc
    _last_run["in_maps"] = in_maps
    _last_run["key"] = key
    res = None
    last_err = None
    for _attempt in range(3):
        try:
            res = run_bass_kernel_spmd(nc, in_maps, list(range(NCORES)))
            break
        except Exception as e:  # noqa: BLE001
            last_err = e
            import time as _time
            _time.sleep(2.0)
    if res is None:
        raise last_err

    counts = np.zeros((B, C), dtype=np.float64)
    sums = np.zeros((B, C), dtype=np.float64)
    jj = np.arange(GROUP)
    for m in range(NCORES):
        o = res.results[m]["out"].astype(np.float64)
        for r in range(R):
            b = m * R + r
            P = o[:, r * 128:(r + 1) * 128]
            if HI_LAYOUT == "strided_ld":
                # stationary cols ordered (t8, h): rows = t8*16+h
                P4 = P.reshape(GROUP, HI, GROUP, 2 * LOB)
                blocks = P4[jj, :, jj, :]          # [8, 16, 16]
                folded = blocks.sum(axis=0)        # [16 h, 16 cv]
            else:
                # stationary cols ordered (h, t8): rows = h*8+t8
                P4 = P.reshape(HI, GROUP, GROUP, 2 * LOB)
                blocks = P4[:, jj, jj, :]          # [16, 8, 16]
                folded = blocks.sum(axis=1)        # [16 h, 16 cv]
            counts[b] = folded[:, :LOB].reshape(C)
            sums[b] = folded[:, LOB:].reshape(C)

    present = counts > 0
    means = np.where(present, sums / np.where(present, counts, 1.0), 0.0)
    pmask = present.astype(np.float64)
    n_clusters_b = pmask.sum(axis=1)
    b_present = n_clusters_b > 0
    batch_loss = (means * pmask).sum(axis=1) / np.where(b_present, n_clusters_b, 1.0)
    n_b = b_present.sum()
    loss = np.where(b_present, batch_loss, 0.0).sum() / max(n_b, 1)
    return np.float32(loss)


def profile_hw(np_inputs=None, k1=4, k2=1004, pairs=10, verbose=False):
    import time
    from concourse.bass_utils import run_bass_kernel_spmd
    if not _last_run and np_inputs is not None:
        kernel(**np_inputs)
    T_pad, R = _last_run["key"]

    ncs = {}
    for k in (k1, k2):
        ck = ("prof", T_pad, R, k, "full")
        if ck not in _prog_cache:
            _prog_cache[ck] = _build_program(T_pad, R, repeat=k,
                                             internal_inputs=True)
        ncs[k] = _prog_cache[ck]

    def one(k):
        t0 = time.time()
        run_bass_kernel_spmd(ncs[k], [{} for _ in range(NCORES)],
                             list(range(NCORES)))
        return time.time() - t0

    one(k1)
    one(k2)
    diffs = []
    for _ in range(pairs):
        try:
            ta = one(k1)
            tb = one(k2)
        except Exception:
            time.sleep(2)
            continue
        diffs.append((tb - ta) / (k2 - k1) * 1e9)
    diffs.sort()
    if verbose:
        print("pair diffs (ns/iter):", [f"{d:.0f}" for d in diffs])
    return diffs[len(diffs) // 2] if diffs else float("nan")


def profile_stages(np_inputs=None, k1=4, k2=404, pairs=8):
    import time
    from concourse.bass_utils import run_bass_kernel_spmd
    if not _last_run and np_inputs is not None:
        kernel(**np_inputs)
    T_pad, R = _last_run["key"]
    out = {}
    stages = ["dma", "dve", "full"]
    for stage in stages:
        ncs = {}
        for k in (k1, k2):
            ck = ("prof", T_pad, R, k, stage)
            if ck not in _prog_cache:
                _prog_cache[ck] = _build_program(T_pad, R, repeat=k,
                                                 internal_inputs=True,
                                                 stage=stage)
            ncs[k] = _prog_cache[ck]

        def one(k):
            t0 = time.time()
            run_bass_kernel_spmd(ncs[k], [{} for _ in range(NCORES)],
                                 list(range(NCORES)))
            return time.time() - t0
        one(k1)
        one(k2)
        diffs = []
        for _ in range(pairs):
            ta = one(k1)
            tb = one(k2)
            diffs.append((tb - ta) / (k2 - k1) * 1e6)
        diffs.sort()
        out[stage] = diffs[len(diffs) // 2]
    return out


# revision 3
# speedup vs baseline: 1.8620x; 1.8223x over previous
"""Trainium2 Bass kernel v2 for nn_AutoEncoderLoss (two-level segment-mean MSE).

Pipeline per [128, tw] tile (points along partitions, t along free dim):
  inputs: rec bf16, tar bf16, lab u8 (5 B/point of HBM traffic)
  DVE:  wb  = bf16(lab + 2076.5)            -> 2080 + 8*(lab>>3)   [TS]
        lb  = (wb - 2080) - lab             -> -(lab & 7)          [STT]
          (or lab mod 8 -> +l when K2_LO=mod)
        d   = rec - tar                                             [TT 2x]
        hi slabs: 16x is_equal(wb, 2080+8H) -> bin-major hic       [TS 4x]
        lo slabs:  8x is_equal(lb, -L)      -> lot[0:8]            [TS 4x]
        val slabs: 8x lot[8+l] = lot[l] * vb                       [TT 2x]
  Act:  vb  = Square(d) -> bf16
  DVE:  hic3 = block-transpose copy of hic -> [g][h][t8] stationary layout
  PE:   per 8-col group g: matmul(psum[r], hic3[g] [128,128], lot[.,g-cols]
        [128, 8t x 16cv]) accumulating counts|sums per (H, L).
Host folds psum diagonal blocks -> [B, C] counts/sums -> masked two-level mean.

Label pad = 255: h=31 matches no hi bin, so pads contribute nothing.
"""

import os as _os
import numpy as np
from contextlib import ExitStack

NCORES = 8
HI = 16
LOB = 8
GROUP = 8
T_TILE = 640
LO_MODE = "stt"
WB_ENGINE = "dve"
HI_LAYOUT = "blk_act"
PAD_LABEL = 255
RHS_MODE = "chunk"

_prog_cache = {}
_last_run = {}


def _build_program(T_pad, R, repeat=None, internal_inputs=False, stage="full"):
    import concourse.tile as tile
    from concourse import bacc, mybir

    f32 = mybir.dt.float32
    bf16 = mybir.dt.bfloat16
    u8 = mybir.dt.uint8
    AT = mybir.ActivationFunctionType
    OP = mybir.AluOpType

    nc = bacc.Bacc("TRN2", target_bir_lowering=False, debug=False,
                   num_devices=NCORES)
    in_kind = "Internal" if internal_inputs else "ExternalInput"
    rec = nc.dram_tensor("rec", [128, R * T_pad], bf16, kind=in_kind).ap()
    tar = nc.dram_tensor("tar", [128, R * T_pad], bf16, kind=in_kind).ap()
    lab = nc.dram_tensor("lab", [128, R * T_pad], u8, kind=in_kind).ap()
    out = nc.dram_tensor("out", [128, R * 128], f32, kind="ExternalOutput").ap()

    tiles = []
    t0 = 0
    while t0 < T_pad:
        tw = min(T_TILE, T_pad - t0)
        tiles.append((t0, tw))
        t0 += tw
    n_mm = T_pad // GROUP

    with tile.TileContext(nc) as tc, ExitStack() as ctx:
        NB = 2
        io_pool = ctx.enter_context(tc.tile_pool(name="io", bufs=NB))
        tmp_pool = ctx.enter_context(tc.tile_pool(name="tmp", bufs=NB))
        slab_pool = ctx.enter_context(tc.tile_pool(name="slab", bufs=NB))
        psum_pool = ctx.enter_context(tc.tile_pool(name="psum", bufs=1, space="PSUM"))
        out_pool = ctx.enter_context(tc.tile_pool(name="outp", bufs=2))

        psums = [psum_pool.tile([128, 128], f32, tag=f"ps{r}", name=f"ps{r}")
                 for r in range(R)] if stage == "full" else [None] * R

        if repeat is not None:
            ctx.enter_context(tc.For_i(0, repeat, 1))

        for r in range(R):
            base = r * T_pad
            mm_i = 0
            for (t0, tw) in tiles:
                gd = tw // GROUP
                rec_t = io_pool.tile([128, tw], bf16, tag="rec")
                nc.sync.dma_start(out=rec_t[:], in_=rec[:, base + t0:base + t0 + tw])
                tar_t = io_pool.tile([128, tw], bf16, tag="tar")
                nc.sync.dma_start(out=tar_t[:], in_=tar[:, base + t0:base + t0 + tw])
                lab_t = io_pool.tile([128, tw], u8, tag="lab")
                nc.sync.dma_start(out=lab_t[:], in_=lab[:, base + t0:base + t0 + tw])
                if stage == "dma":
                    continue
                # cumulative sub-stages for in-situ cost attribution:
                # dma < pre < hi < lo < mul < rep(=dve) < full
                want = {"pre": 1, "hi": 2, "lo": 3, "mul": 4, "rep": 5,
                        "dve": 5, "full": 9}[stage]

                # wb = 2080 + 8h  (bf16 RN in [2048,4096) floors to x8 grid)
                wb = tmp_pool.tile([128, tw], bf16, tag="wb")
                if WB_ENGINE == "act":
                    nc.scalar.activation(wb[:], lab_t[:], AT.Identity,
                                         bias=2076.5, scale=1.0)
                else:
                    nc.vector.tensor_scalar(wb[:], lab_t[:], 2076.5, None, OP.add)
                # lb = -(l) via STT
                lb = tmp_pool.tile([128, tw], bf16, tag="lb")
                nc.vector.scalar_tensor_tensor(lb[:], wb[:], -2080.0,
                                               lab_t[:], OP.add, OP.subtract)
                lo_const = lambda l: float(-l)
                # d and vb = d^2
                d_t = tmp_pool.tile([128, tw], bf16, tag="d")
                nc.vector.tensor_sub(d_t[:], rec_t[:], tar_t[:])
                vb = tmp_pool.tile([128, tw], bf16, tag="vb")
                nc.scalar.activation(vb[:], d_t[:], AT.Square)
                if want < 2:
                    continue

                # hi slabs
                if HI_LAYOUT == "strided_ld":
                    hic = slab_pool.tile([128, HI, tw], bf16, tag="hic")
                    for h in range(HI):
                        nc.vector.tensor_scalar(hic[:, h, :], wb[:],
                                                2080.0 + 8 * h, None, OP.is_equal)
                    lhs_of = lambda g: hic[:, :, g * GROUP:(g + 1) * GROUP]
                else:
                    hic = slab_pool.tile([128, HI, tw], bf16, tag="hic")
                    for h in range(HI):
                        nc.vector.tensor_scalar(hic[:, h, :], wb[:],
                                                2080.0 + 8 * h, None, OP.is_equal)
                if want < 3:
                    continue

                # lo slabs: counts
                lot = slab_pool.tile([128, 2 * LOB, tw], bf16, tag="lot")
                for l in range(LOB):
                    nc.vector.tensor_scalar(lot[:, l, :], lb[:], lo_const(l),
                                            None, OP.is_equal)
                if want < 4:
                    continue
                # val slabs
                for l in range(LOB):
                    nc.vector.tensor_mul(lot[:, LOB + l, :], lot[:, l, :], vb[:])
                if want < 5:
                    continue

                # stationary repack (unless strided_ld reads hic directly)
                if HI_LAYOUT != "strided_ld":
                    hic3 = slab_pool.tile([128, gd * HI * GROUP], bf16, tag="hic3")
                    src = hic[:].rearrange("p h (g e) -> p g h e", e=GROUP)
                    dst = hic3[:].rearrange("p (g h e) -> p g h e",
                                            g=gd, h=HI, e=GROUP)
                    if HI_LAYOUT == "blk_act":
                        nc.scalar.copy(dst, src)
                    else:
                        nc.vector.tensor_copy(dst, src)
                    h3v = hic3[:].rearrange("p (g h e) -> p g h e",
                                            g=gd, h=HI, e=GROUP)
                    lhs_of = lambda g: h3v[:, g, :, :]

                if want < 9:
                    continue

                for g in range(gd):
                    if RHS_MODE == "chunk":
                        rhs = lot[:, :, g * GROUP:(g + 1) * GROUP]
                    else:
                        rhs = lot[:, :, g * GROUP:(g + 1) * GROUP].transpose([0, 2, 1])
                    nc.tensor.matmul(psums[r][:], lhs_of(g), rhs,
                                     start=(mm_i == 0), stop=(mm_i == n_mm - 1))
                    mm_i += 1

            if stage == "full":
                ob = out_pool.tile([128, 128], f32, tag="ob")
                nc.vector.tensor_copy(ob[:], psums[r][:])
                nc.sync.dma_start(out=out[:, r * 128:(r + 1) * 128], in_=ob[:])

    nc.compile()
    return nc


def kernel(reco, target, clabel, batch_index, num_batches, num_clusters):
    import ml_dtypes
    from concourse.bass_utils import run_bass_kernel_spmd

    B = int(num_batches)
    C = int(num_clusters)
    assert C == HI * LOB, f"kernel hardcoded for 128 clusters, got {C}"
    assert B % NCORES == 0, f"num_batches {B} not divisible by {NCORES}"
    R = B // NCORES

    reco = np.ascontiguousarray(np.asarray(reco, dtype=np.float32).reshape(-1))
    target = np.ascontiguousarray(np.asarray(target, dtype=np.float32).reshape(-1))
    clabel = np.asarray(clabel).astype(np.uint8).reshape(-1)
    batch_index = np.asarray(batch_index).reshape(-1)
    N = reco.shape[0]

    bnd = np.searchsorted(batch_index, np.arange(B + 1), side="left")
    t_len = (np.diff(bnd) + 127) // 128
    T_pad = int(-(-int(t_len.max()) // GROUP) * GROUP)
    T_pad = max(T_pad, GROUP)

    key = (T_pad, R)
    if key not in _prog_cache:
        _prog_cache[key] = _build_program(T_pad, R)
    nc = _prog_cache[key]

    in_maps = []
    for m in range(NCORES):
        rec_buf = np.zeros((128, R * T_pad), dtype=ml_dtypes.bfloat16)
        tar_buf = np.zeros((128, R * T_pad), dtype=ml_dtypes.bfloat16)
        lab_buf = np.full((128, R * T_pad), PAD_LABEL, dtype=np.uint8)
        for r in range(R):
            b = m * R + r
            s, e = int(bnd[b]), int(bnd[b + 1])
            n = e - s
            if n == 0:
                continue
            tpb = (n + 127) // 128
            block = np.zeros(128 * tpb, dtype=np.float32)
            block[:n] = reco[s:e]
            rec_buf[:, r * T_pad:r * T_pad + tpb] = \
                block.astype(ml_dtypes.bfloat16).reshape(128, tpb)
            block = np.zeros(128 * tpb, dtype=np.float32)
            block[:n] = target[s:e]
            tar_buf[:, r * T_pad:r * T_pad + tpb] = \
                block.astype(ml_dtypes.bfloat16).reshape(128, tpb)
            lblock = np.full(128 * tpb, PAD_LABEL, dtype=np.uint8)
            lblock[:n] = clabel[s:e]
            lab_buf[:, r * T_pad:r * T_pad + tpb] = lblock.reshape(128, tpb)
        in_maps.append({"rec": rec_buf, "tar": tar_buf, "lab": lab_buf})

    _last_run["nc"] = nc
    _last_run["in_maps"] = in_maps
    _last_run["key"] = key
    res = None
    last_err = None
    for _attempt in range(3):
        try:
            res = run_bass_kernel_spmd(nc, in_maps, list(range(NCORES)))
            break
        except Exception as e:  # noqa: BLE001
            last_err = e
            import time as _time
            _time.sleep(2.0)
    if res is None:
        raise last_err

    counts = np.zeros((B, C), dtype=np.float64)
    sums = np.zeros((B, C), dtype=np.float64)
    jj = np.arange(GROUP)
    for m in range(NCORES):
        o = res.results[m]["out"].astype(np.float64)
        for r in range(R):
            b = m * R + r
            P = o[:, r * 128:(r + 1) * 128]
            if RHS_MODE == "chunk":
                # rows (h,t8), cols (cv,t): diag over (t8, t)
                P4 = P.reshape(HI, GROUP, 2 * LOB, GROUP)
                blocks = P4[:, jj, :, jj]          # [8, 16, 16]
                folded = blocks.sum(axis=0)        # [16 h, 16 cv]
            else:
                # rows (h,t8), cols (t,cv)
                P4 = P.reshape(HI, GROUP, GROUP, 2 * LOB)
                blocks = P4[:, jj, jj, :]          # [16, 8, 16]
                folded = blocks.sum(axis=1)        # [16 h, 16 cv]
            counts[b] = folded[:, :LOB].reshape(C)
            sums[b] = folded[:, LOB:].reshape(C)

    present = counts > 0
    means = np.where(present, sums / np.where(present, counts, 1.0), 0.0)
    pmask = present.astype(np.float64)
    n_clusters_b = pmask.sum(axis=1)
    b_present = n_clusters_b > 0
    batch_loss = (means * pmask).sum(axis=1) / np.where(b_present, n_clusters_b, 1.0)
    n_b = b_present.sum()
    loss = np.where(b_present, batch_loss, 0.0).sum() / max(n_b, 1)
    return np.float32(loss)


def profile_hw(np_inputs=None, k1=4, k2=1004, pairs=10, verbose=False):
    import time
    from concourse.bass_utils import run_bass_kernel_spmd
    if not _last_run and np_inputs is not None:
        kernel(**np_inputs)
    T_pad, R = _last_run["key"]

    ncs = {}
    for k in (k1, k2):
        ck = ("prof", T_pad, R, k, "full")
        if ck not in _prog_cache:
            _prog_cache[ck] = _build_program(T_pad, R, repeat=k,
                                             internal_inputs=True)
        ncs[k] = _prog_cache[ck]

    def one(k):
        t0 = time.time()
        run_bass_kernel_spmd(ncs[k], [{} for _ in range(NCORES)],
                             list(range(NCORES)))
        return time.time() - t0

    one(k1)
    one(k2)
    diffs = []
    for _ in range(pairs):
        try:
            ta = one(k1)
            tb = one(k2)
        except Exception:
            time.sleep(2)
            continue
        diffs.append((tb - ta) / (k2 - k1) * 1e9)
    diffs.sort()
    if verbose:
        print("pair diffs (ns/iter):", [f"{d:.0f}" for d in diffs])
    return diffs[len(diffs) // 2] if diffs else float("nan")


def profile_stages(np_inputs=None, k1=4, k2=404, pairs=8):
    import time
    from concourse.bass_utils import run_bass_kernel_spmd
    if not _last_run and np_inputs is not None:
        kernel(**np_inputs)
    T_pad, R = _last_run["key"]
    out = {}
    stages = ["dma", "dve", "full"]
    for stage in stages:
        ncs = {}
        for k in (k1, k2):
            ck = ("prof", T_pad, R, k, stage)
            if ck not in _prog_cache:
                _prog_cache[ck] = _build_program(T_pad, R, repeat=k,
                                                 internal_inputs=True,
                                                 stage=stage)
            ncs[k] = _prog_cache[ck]

        def one(k):
            t0 = time.time()
            run_bass_kernel_spmd(ncs[k], [{} for _ in range(NCORES)],
                                 list(range(NCORES)))
            return time.time() - t0
        one(k1)
        one(k2)
        diffs = []
        for _ in range(pairs):
            ta = one(k1)
            tb = one(k2)
            diffs.append((tb - ta) / (k2 - k1) * 1e6)
        diffs.sort()
        out[stage] = diffs[len(diffs) // 2]
    return out
